# revision 12
# baseline (speedup 1.0000x reference)
"""Trainium2 Bass kernel for a 2-layer LSTM (B=2048, T=512, IN=2, H=64) + FC head.

Data-parallel over 8 NeuronCores: batch 2048 -> 256 per core, weights replicated.

On-chip layout (per core, B_local = 256):
  - Hidden/cell state of BOTH layers packed in one [128, 256] SBUF tile:
    rows 0:64 = layer0 h (or c), rows 64:128 = layer1 h (or c); free dim = batch.
  - Software pipeline: iteration i computes layer0 @ t=i and layer1 @ t=i-1.
  - PSUM is split into TWO tiles per step (each [128, 2*512] = 2 banks,
    double-buffered => all 8 banks): pfi holds gates (f, i), pgo holds (g, o).
    Per-gate-pair tiles make the Tile dependency tracker release the f,i
    sigmoid after only the f and i recurrent matmuls (not all four), and let
    the next step's x-inject matmuls start as soon as the two-steps-back
    activations have read their banks.
  - Per step the PE runs: [x-inject(t+1) K=3 MMs x4] then [recurrent K=128
    MMs x4]. The x-injects are issued BEFORE the recurrent MMs in the PE
    FIFO: they have no dependency on h(t), so they execute during the
    activation/vector tail of step t, keeping the PE dense.
  - Biases ride the ones-row of the x-inject matmul (w0xb).
  - VectorE: c = f*c + i*g ; h = o * tanh(c)  (tanh/sigmoid on ScalarE).
  - Matmul operands are bf16; PSUM fp32; activations/cell state bf16.
"""

import numpy as np
from contextlib import contextmanager

B, T, IN, H, NCLS = 2048, 512, 2, 64, 9
NCORES = 8
BL = B // NCORES          # 256 batch per core
TC = 16                   # timesteps per x chunk DMA

import os as _os

MM_BF16 = _os.environ.get("MM_BF16", "1") == "1"    # bf16 matmul operands
ACT_BF16 = _os.environ.get("ACT_BF16", "1") == "1"  # bf16 gate activations
C_BF16 = _os.environ.get("C_BF16", "1") == "1"      # bf16 cell state
XR = 3  # x rows: x0, x1, ones

LAST_EXEC_NS = None
LAST_TRACE = None

_CACHE = {}


def _np_mmdt():
    if MM_BF16:
        import ml_dtypes
        return ml_dtypes.bfloat16
    return np.float32


def _build(steps=T):
    import concourse.bacc as bacc
    import concourse.tile as tile
    from concourse import mybir
    from contextlib import ExitStack

    f32 = mybir.dt.float32
    mdt = mybir.dt.bfloat16 if MM_BF16 else f32
    adt = mybir.dt.bfloat16 if ACT_BF16 else f32
    cdt = mybir.dt.bfloat16 if C_BF16 else f32
    AF = mybir.ActivationFunctionType
    SIG, TANH = AF.Sigmoid, AF.Tanh

    nc = bacc.Bacc(None, target_bir_lowering=False)

    x_d = nc.dram_tensor("x", [XR, steps * BL], mdt, kind="ExternalInput")
    w1_d = nc.dram_tensor("w1", [2 * H, 4 * H], mdt, kind="ExternalInput")   # [w_ih1;w_hh1].T
    wc_d = nc.dram_tensor("wc", [2 * H, 8 * H], mdt, kind="ExternalInput")   # fused per-gate [128,128]
    w0xb_d = nc.dram_tensor("w0xb", [XR, 8 * H], mdt, kind="ExternalInput")  # x-inject+bias per-gate [3,128]
    bias_d = nc.dram_tensor("bias", [2 * H, 4], f32, kind="ExternalInput")
    wfc_d = nc.dram_tensor("wfc", [2 * H, NCLS], mdt, kind="ExternalInput")  # top half zeros
    bfc_d = nc.dram_tensor("bfc", [NCLS, 1], f32, kind="ExternalInput")
    out_d = nc.dram_tensor("out", [NCLS, BL], f32, kind="ExternalOutput")

    with tile.TileContext(nc) as tc, ExitStack() as ctx:
        consts = ctx.enter_context(tc.tile_pool(name="consts", bufs=1))
        state = ctx.enter_context(tc.tile_pool(name="state", bufs=1))
        xpool = ctx.enter_context(tc.tile_pool(name="xchunk", bufs=2))
        acts = ctx.enter_context(tc.tile_pool(name="acts", bufs=2))
        psum = ctx.enter_context(tc.tile_pool(name="psum", bufs=3, space="PSUM"))

        def load_const(shape, dt, dram, tag):
            t = consts.tile(shape, dt, tag=tag)
            nc.sync.dma_start(t[:], dram[:, :])
            return t

        w1 = load_const([2 * H, 4 * H], mdt, w1_d, "w1")
        wc = load_const([2 * H, 8 * H], mdt, wc_d, "wc")
        w0xb = load_const([XR, 8 * H], mdt, w0xb_d, "w0xb")
        bias = load_const([2 * H, 4], f32, bias_d, "bias")
        wfc = load_const([2 * H, NCLS], mdt, wfc_d, "wfc")
        bfc = load_const([NCLS, 1], f32, bfc_d, "bfc")

        h_all = state.tile([2 * H, BL], mdt)
        c_all = state.tile([2 * H, BL], cdt)
        nc.vector.memset(h_all[:], 0.0)
        nc.vector.memset(c_all[:], 0.0)

        # gate order in the 4H weight dim (PyTorch): i, f, g, o
        GI, GF, GG, GO = 0, 1, 2, 3
        gsl = lambda j: slice(j * H, (j + 1) * H)
        BW = 512
        # One PSUM bank (512 f32) per gate PAIR: pfi = [f | i], pgo = [g | o],
        # each [128, 512] with the two gates at cols 0:256 / 256:512. The
        # pair's first x-inject uses start=True (clears the whole bank); the
        # second uses start=False, which OVERWRITES its freshly-cleared region
        # (has_written=0) rather than accumulating. Recurrent MMs then
        # accumulate (has_written=1). Contiguous pairs make SIGfi a plain 2D
        # FD=512 read, and 1-bank tiles allow bufs=4 (8 banks total) for
        # deeper x-inject lookahead.
        GLOC = {GF: (0, 0, True), GI: (0, BL, False),
                GG: (1, 0, True), GO: (1, BL, False)}

        xch = None
        cur = None          # psum tiles (pfi, pgo) for the current step
        nxt = None          # psum tiles pre-written with x-inject for step t+1

        # HAM warm-up + per-step fillers: the PE clock gate (HAM) throttles to
        # 1.2 GHz unless the PE stays busy; a ~3us dense burst flips it to
        # 2.4 GHz (measured: matmul spacing 107ns -> 56ns after ~25 MMs), and
        # per-step filler matmuls into a scratch bank keep it from
        # re-throttling between recurrent bursts. Warm recurrent MMs save
        # ~240ns/step on the serial chain.
        NWARM = int(_os.environ.get("WARMUP", "120"))
        NFILL = int(_os.environ.get("FILLERS", "12"))
        scratch = None
        if NWARM or NFILL:
            scratch = psum.tile([2 * H, BW], f32, tag="scratch", bufs=1)
        for _ in range(NWARM):
            nc.tensor.matmul(scratch[:, 0:128], lhsT=wc[:, 0:128],
                             rhs=wc[:, 0:128], start=True, stop=True,
                             skip_group_check=True)

        def xinj(t, xt):
            """Allocate psum tiles for step t and run the 4 x-inject MMs."""
            pfi = psum.tile([2 * H, BW], f32, tag="pfi")
            pgo = psum.tile([2 * H, BW], f32, tag="pgo")
            tiles = (pfi, pgo)
            for j in (GF, GI, GG, GO):
                ti, co, st = GLOC[j]
                nc.tensor.matmul(
                    tiles[ti][:, co : co + BL],
                    lhsT=w0xb[:, 2 * H * j : 2 * H * (j + 1)],
                    rhs=xt, start=st, stop=False, skip_group_check=True,
                )
            return tiles

        def xt_of(t):
            return xch[:, (t % TC) * BL : (t % TC + 1) * BL]

        for it in range(steps + 1):
            do0 = it < steps

            if it == 0:
                tc_cur = min(TC, steps)
                xch = xpool.tile([XR, TC * BL], mdt)
                nc.sync.dma_start(
                    xch[:, : tc_cur * BL], x_d[:, 0 : tc_cur * BL]
                )
                nxt = xinj(0, xt_of(0))

            cur, nxt = nxt, None

            if do0:
                # current step's gate matmuls first need h(t); meanwhile the
                # next step's x-injects (below, issued first in FIFO) run.
                if it + 1 < steps:
                    if (it + 1) % TC == 0:
                        tc_nx = min(TC, steps - (it + 1))
                        xch = xpool.tile([XR, TC * BL], mdt)
                        nc.sync.dma_start(
                            xch[:, : tc_nx * BL],
                            x_d[:, (it + 1) * BL : (it + 1 + tc_nx) * BL],
                        )
                    nxt = xinj(it + 1, xt_of(it + 1))
                # recurrent + layer1-inject: one K=128 MM per gate
                for j in (GF, GI, GG, GO):
                    ti, co, _ = GLOC[j]
                    nc.tensor.matmul(
                        cur[ti][:, co : co + BL],
                        lhsT=wc[:, 2 * H * j : 2 * H * (j + 1)],
                        rhs=h_all[:, :], start=False, stop=True,
                        skip_group_check=True,
                    )
                for _ in range(NFILL):
                    nc.tensor.matmul(scratch[:, 0:BL], lhsT=wc[:, 0:128],
                                     rhs=wc[:, 0:BL], start=True, stop=True,
                                     skip_group_check=True)
                lo, hi = 0, (2 * H if it >= 1 else H)
            else:
                # final iteration: layer1 only @ t = steps-1
                pfi = psum.tile([2 * H, BW], f32, tag="pfi")
                pgo = psum.tile([2 * H, BW], f32, tag="pgo")
                cur = (pfi, pgo)
                for j in (GF, GI, GG, GO):
                    ti, co, st = GLOC[j]
                    nc.tensor.matmul(
                        cur[ti][H : 2 * H, co : co + BL],
                        lhsT=w1[:, gsl(j)], rhs=h_all[:, :],
                        start=st, stop=True, skip_group_check=True,
                    )
                lo, hi = H, 2 * H

            sl = slice(lo, hi)
            bias_kw = {}
            if not do0:
                # biases normally ride the x-inject ones-row; the final
                # L1-only step has no x-inject, so use the ACT bias operand.
                bias_kw = {GF: dict(bias=bias[sl, GF : GF + 1]),
                           GI: dict(bias=bias[sl, GI : GI + 1]),
                           GG: dict(bias=bias[sl, GG : GG + 1]),
                           GO: dict(bias=bias[sl, GO : GO + 1])}

            s_fi = acts.tile([2 * H, 2 * BL], adt, tag="sfi")
            if do0:
                nc.scalar.activation(s_fi[sl, :], cur[0][sl, :], SIG)
            else:
                nc.scalar.activation(s_fi[sl, 0:BL], cur[0][sl, 0:BL], SIG,
                                     **bias_kw[GF])
                nc.scalar.activation(s_fi[sl, BL : 2 * BL], cur[0][sl, BL : 2 * BL],
                                     SIG, **bias_kw[GI])
            s_g = acts.tile([2 * H, BL], adt, tag="sg")
            nc.scalar.activation(s_g[sl, :], cur[1][sl, 0:BL], TANH,
                                 **(bias_kw.get(GG, {}) if not do0 else {}))
            s_o = acts.tile([2 * H, BL], adt, tag="so")
            nc.scalar.activation(s_o[sl, :], cur[1][sl, BL : 2 * BL], SIG,
                                 **(bias_kw.get(GO, {}) if not do0 else {}))
            f_ap, i_ap = s_fi[:, 0:BL], s_fi[:, BL : 2 * BL]

            t_fc = acts.tile([2 * H, BL], cdt, tag="tfc")
            nc.vector.tensor_mul(t_fc[sl, :], f_ap[sl, :], c_all[sl, :])
            t_ig = acts.tile([2 * H, BL], cdt, tag="tig")
            nc.vector.tensor_mul(t_ig[sl, :], i_ap[sl, :], s_g[sl, :])
            nc.vector.tensor_add(c_all[sl, :], t_fc[sl, :], t_ig[sl, :])
            s_tc = acts.tile([2 * H, BL], adt, tag="stc")
            nc.scalar.activation(s_tc[sl, :], c_all[sl, :], TANH)
            nc.vector.tensor_mul(h_all[sl, :], s_o[sl, :], s_tc[sl, :])

        # FC head on h2(T-1) = h_all[64:128]; wfc is zero-padded on rows 0:64
        p_fc = psum.tile([2 * H, BW], f32, tag="pfi")
        nc.tensor.matmul(p_fc[0:NCLS, 0:BL], lhsT=wfc[:, :], rhs=h_all[:, :],
                         start=True, stop=True)
        o_sb = acts.tile([2 * H, BL], f32, tag="osb")
        nc.vector.tensor_scalar_add(o_sb[0:NCLS, :], p_fc[0:NCLS, 0:BL], bfc[:, 0:1])
        nc.sync.dma_start(out_d[:, :], o_sb[0:NCLS, :])

    nc.finalize()
    return nc


def _prep_weights(w_ih0, w_hh0, b_ih0, b_hh0, w_ih1, w_hh1, b_ih1, b_hh1, w_fc, b_fc):
    f = np.float32
    mdt = _np_mmdt()
    w1 = np.ascontiguousarray(
        np.concatenate([np.asarray(w_ih1), np.asarray(w_hh1)], 1).T
    ).astype(mdt)                                                        # [128, 256]
    # fused per-gate [K=128, M=128] blocks: cols 0:64 -> layer0 gate (zeros on
    # h2 rows), cols 64:128 -> layer1 gate ([w_ih1; w_hh1])
    wcf = np.zeros((2 * H, 8 * H), dtype=np.float32)
    w1f = np.concatenate([np.asarray(w_ih1), np.asarray(w_hh1)], 1)  # [256, 128]
    for g in range(4):
        wcf[0:H, 2 * H * g : 2 * H * g + H] = np.asarray(w_hh0)[g * H:(g + 1) * H, :].T
        wcf[:, 2 * H * g + H : 2 * H * (g + 1)] = w1f[g * H:(g + 1) * H, :].T
    wc = np.ascontiguousarray(wcf).astype(mdt)
    b0v = (np.asarray(b_ih0) + np.asarray(b_hh0)).astype(np.float32)
    b1v = (np.asarray(b_ih1) + np.asarray(b_hh1)).astype(np.float32)
    w0xbf = np.zeros((XR, 8 * H), dtype=np.float32)
    for g in range(4):
        w0xbf[0:IN, 2 * H * g : 2 * H * g + H] = \
            np.asarray(w_ih0)[g * H:(g + 1) * H, :].T
        w0xbf[IN, 2 * H * g : 2 * H * g + H] = b0v[g * H:(g + 1) * H]
        w0xbf[IN, 2 * H * g + H : 2 * H * (g + 1)] = b1v[g * H:(g + 1) * H]
    w0xb = np.ascontiguousarray(w0xbf).astype(mdt)
    b0 = (np.asarray(b_ih0) + np.asarray(b_hh0)).astype(f).reshape(4, H)
    b1 = (np.asarray(b_ih1) + np.asarray(b_hh1)).astype(f).reshape(4, H)
    bias = np.ascontiguousarray(np.concatenate([b0.T, b1.T], axis=0), dtype=f)
    wfc = np.zeros((2 * H, NCLS), dtype=f)
    wfc[H:, :] = np.asarray(w_fc).T
    wfc = wfc.astype(mdt)
    bfc = np.ascontiguousarray(np.asarray(b_fc).reshape(NCLS, 1), dtype=f)
    return dict(w1=w1, wc=wc, w0xb=w0xb, bias=bias, wfc=wfc, bfc=bfc)


def _prep_x(x, steps=T):
    mdt = _np_mmdt()
    x = np.asarray(x, dtype=np.float32)
    per_core = []
    for c in range(NCORES):
        xc = x[c * BL : (c + 1) * BL, :steps, :]          # [BL, steps, IN]
        xc = xc.transpose(2, 1, 0).reshape(IN, steps * BL)  # [IN, steps*BL]
        xa = np.ones((XR, steps * BL), dtype=np.float32)
        xa[0:IN] = xc
        per_core.append(np.ascontiguousarray(xa).astype(mdt))
    return per_core


@contextmanager
def _fast_compile():
    """Disable walrus birsim (compile-time BIR simulation): it costs ~7s per
    LSTM step (~1h for T=512) and only re-verifies what CoreSim already
    checked. NEFF output is identical."""
    import concourse.bass_utils as bu

    orig = bu.run_command

    def patched(argv, **kw):
        argv = [
            a.replace("--enable-birsim=true", "--enable-birsim=false")
            if isinstance(a, str) else a
            for a in argv
        ]
        return orig(argv, **kw)

    bu.run_command = patched
    try:
        yield
    finally:
        bu.run_command = orig


def kernel(x, w_ih0, w_hh0, b_ih0, b_hh0, w_ih1, w_hh1, b_ih1, b_hh1,
           w_fc, b_fc, _steps=T, _trace=False):
    global LAST_EXEC_NS, LAST_TRACE
    from concourse.bass_utils import run_bass_kernel_spmd

    key = ("nc", _steps)
    if key not in _CACHE:
        _CACHE[key] = _build(steps=_steps)
    nc = _CACHE[key]

    wmap = _prep_weights(w_ih0, w_hh0, b_ih0, b_hh0,
                         w_ih1, w_hh1, b_ih1, b_hh1, w_fc, b_fc)
    xs = _prep_x(x, _steps)
    in_maps = [{"x": xs[c], **wmap} for c in range(NCORES)]

    with _fast_compile():
        res = run_bass_kernel_spmd(nc, in_maps, core_ids=list(range(NCORES)),
                                   trace=_trace)
    LAST_EXEC_NS = res.exec_time_ns
    LAST_TRACE = res.instructions_and_trace
    out = np.concatenate([r["out"].T for r in res.results], axis=0)  # [B, 9]
    return out.astype(np.float32)


# revision 14
# speedup vs baseline: 1.0258x; 1.0258x over previous
"""Trainium2 Bass kernel for a 2-layer LSTM (B=2048, T=512, IN=2, H=64) + FC head.

Data-parallel over 8 NeuronCores: batch 2048 -> 256 per core, weights replicated.

On-chip layout (per core, B_local = 256):
  - Hidden/cell state of BOTH layers packed in one [128, 256] SBUF tile:
    rows 0:64 = layer0 h (or c), rows 64:128 = layer1 h (or c); free dim = batch.
  - Software pipeline: iteration i computes layer0 @ t=i and layer1 @ t=i-1.
  - PSUM is split into TWO tiles per step (each [128, 2*512] = 2 banks,
    double-buffered => all 8 banks): pfi holds gates (f, i), pgo holds (g, o).
    Per-gate-pair tiles make the Tile dependency tracker release the f,i
    sigmoid after only the f and i recurrent matmuls (not all four), and let
    the next step's x-inject matmuls start as soon as the two-steps-back
    activations have read their banks.
  - Per step the PE runs: [x-inject(t+1) K=3 MMs x4] then [recurrent K=128
    MMs x4]. The x-injects are issued BEFORE the recurrent MMs in the PE
    FIFO: they have no dependency on h(t), so they execute during the
    activation/vector tail of step t, keeping the PE dense.
  - Biases ride the ones-row of the x-inject matmul (w0xb).
  - VectorE: c = f*c + i*g ; h = o * tanh(c)  (tanh/sigmoid on ScalarE).
  - Matmul operands are bf16; PSUM fp32; activations/cell state bf16.
"""

import numpy as np
from contextlib import contextmanager

B, T, IN, H, NCLS = 2048, 512, 2, 64, 9
NCORES = 8
BL = B // NCORES          # 256 batch per core
TC = 16                   # timesteps per x chunk DMA

import os as _os

MM_BF16 = _os.environ.get("MM_BF16", "1") == "1"    # bf16 matmul operands
ACT_BF16 = _os.environ.get("ACT_BF16", "1") == "1"  # bf16 gate activations
C_BF16 = _os.environ.get("C_BF16", "1") == "1"      # bf16 cell state
XR = 3  # x rows: x0, x1, ones

LAST_EXEC_NS = None
LAST_TRACE = None

_CACHE = {}


def _np_mmdt():
    if MM_BF16:
        import ml_dtypes
        return ml_dtypes.bfloat16
    return np.float32


def _build(steps=T):
    import concourse.bacc as bacc
    import concourse.tile as tile
    from concourse import mybir
    from contextlib import ExitStack

    f32 = mybir.dt.float32
    mdt = mybir.dt.bfloat16 if MM_BF16 else f32
    adt = mybir.dt.bfloat16 if ACT_BF16 else f32
    cdt = mybir.dt.bfloat16 if C_BF16 else f32
    AF = mybir.ActivationFunctionType
    SIG, TANH = AF.Sigmoid, AF.Tanh

    nc = bacc.Bacc(None, target_bir_lowering=False)

    x_d = nc.dram_tensor("x", [XR, steps * BL], mdt, kind="ExternalInput")
    w1_d = nc.dram_tensor("w1", [2 * H, 4 * H], mdt, kind="ExternalInput")   # [w_ih1;w_hh1].T
    wc_d = nc.dram_tensor("wc", [2 * H, 8 * H], mdt, kind="ExternalInput")   # fused per-gate [128,128]
    w0xb_d = nc.dram_tensor("w0xb", [XR, 8 * H], mdt, kind="ExternalInput")  # x-inject+bias per-gate [3,128]
    bias_d = nc.dram_tensor("bias", [2 * H, 4], f32, kind="ExternalInput")
    wfc_d = nc.dram_tensor("wfc", [2 * H, NCLS], mdt, kind="ExternalInput")  # top half zeros
    bfc_d = nc.dram_tensor("bfc", [NCLS, 1], f32, kind="ExternalInput")
    out_d = nc.dram_tensor("out", [NCLS, BL], f32, kind="ExternalOutput")

    with tile.TileContext(nc) as tc, ExitStack() as ctx:
        consts = ctx.enter_context(tc.tile_pool(name="consts", bufs=1))
        state = ctx.enter_context(tc.tile_pool(name="state", bufs=1))
        xpool = ctx.enter_context(tc.tile_pool(name="xchunk", bufs=2))
        acts = ctx.enter_context(tc.tile_pool(name="acts", bufs=2))
        NWARM = int(_os.environ.get("WARMUP", "0"))
        NFILL = int(_os.environ.get("FILLERS", "0"))
        # 8 PSUM banks: pfi + pgo tags, [128, 512] (1 bank) each, 4-deep
        # rotation -> x-injects for step t+4 can run as soon as step t's
        # activations have read their banks. (3-deep + 1 scratch bank when
        # the HAM warmup/filler experiment is enabled.)
        psum = ctx.enter_context(tc.tile_pool(
            name="psum", bufs=(3 if (NWARM or NFILL) else 4), space="PSUM"))

        def load_const(shape, dt, dram, tag):
            t = consts.tile(shape, dt, tag=tag)
            nc.sync.dma_start(t[:], dram[:, :])
            return t

        w1 = load_const([2 * H, 4 * H], mdt, w1_d, "w1")
        wc = load_const([2 * H, 8 * H], mdt, wc_d, "wc")
        w0xb = load_const([XR, 8 * H], mdt, w0xb_d, "w0xb")
        bias = load_const([2 * H, 4], f32, bias_d, "bias")
        wfc = load_const([2 * H, NCLS], mdt, wfc_d, "wfc")
        bfc = load_const([NCLS, 1], f32, bfc_d, "bfc")

        h_all = state.tile([2 * H, BL], mdt)
        c_all = state.tile([2 * H, BL], cdt)
        nc.vector.memset(h_all[:], 0.0)
        nc.vector.memset(c_all[:], 0.0)

        # gate order in the 4H weight dim (PyTorch): i, f, g, o
        GI, GF, GG, GO = 0, 1, 2, 3
        gsl = lambda j: slice(j * H, (j + 1) * H)
        BW = 512
        # One PSUM bank (512 f32) per gate PAIR: pfi = [f | i], pgo = [g | o],
        # each [128, 512] with the two gates at cols 0:256 / 256:512. The
        # pair's first x-inject uses start=True (clears the whole bank); the
        # second uses start=False, which OVERWRITES its freshly-cleared region
        # (has_written=0) rather than accumulating. Recurrent MMs then
        # accumulate (has_written=1). Contiguous pairs make SIGfi a plain 2D
        # FD=512 read, and 1-bank tiles allow bufs=4 (8 banks total) for
        # deeper x-inject lookahead.
        GLOC = {GF: (0, 0, True), GI: (0, BL, False),
                GG: (1, 0, True), GO: (1, BL, False)}

        xch = None
        cur = None          # psum tiles (pfi, pgo) for the current step
        nxt = None          # psum tiles pre-written with x-inject for step t+1

        # HAM warm-up + per-step fillers (experiment, off by default): a ~3us
        # dense MM burst flips the PE clock gate 1.2 -> 2.4 GHz (measured:
        # spacing 107 -> 56ns after ~25 MMs) and fillers try to keep it warm.
        # Measured net-NEGATIVE: the fillers head-of-line-block the recurrent
        # MMs in the PE FIFO (+120ns/step) and HAM re-throttles anyway.
        scratch = None
        if NWARM or NFILL:
            scratch = psum.tile([2 * H, BW], f32, tag="scratch", bufs=1)
        for _ in range(NWARM):
            nc.tensor.matmul(scratch[:, 0:128], lhsT=wc[:, 0:128],
                             rhs=wc[:, 0:128], start=True, stop=True,
                             skip_group_check=True)

        def xinj(t, xt):
            """Allocate psum tiles for step t and run the 4 x-inject MMs."""
            pfi = psum.tile([2 * H, BW], f32, tag="pfi")
            pgo = psum.tile([2 * H, BW], f32, tag="pgo")
            tiles = (pfi, pgo)
            for j in (GF, GI, GG, GO):
                ti, co, st = GLOC[j]
                nc.tensor.matmul(
                    tiles[ti][:, co : co + BL],
                    lhsT=w0xb[:, 2 * H * j : 2 * H * (j + 1)],
                    rhs=xt, start=st, stop=False, skip_group_check=True,
                )
            return tiles

        def xt_of(t):
            return xch[:, (t % TC) * BL : (t % TC + 1) * BL]

        for it in range(steps + 1):
            do0 = it < steps

            if it == 0:
                tc_cur = min(TC, steps)
                xch = xpool.tile([XR, TC * BL], mdt)
                nc.sync.dma_start(
                    xch[:, : tc_cur * BL], x_d[:, 0 : tc_cur * BL]
                )
                nxt = xinj(0, xt_of(0))

            cur, nxt = nxt, None

            if do0:
                # current step's gate matmuls first need h(t); meanwhile the
                # next step's x-injects (below, issued first in FIFO) run.
                if it + 1 < steps:
                    if (it + 1) % TC == 0:
                        tc_nx = min(TC, steps - (it + 1))
                        xch = xpool.tile([XR, TC * BL], mdt)
                        nc.sync.dma_start(
                            xch[:, : tc_nx * BL],
                            x_d[:, (it + 1) * BL : (it + 1 + tc_nx) * BL],
                        )
                    nxt = xinj(it + 1, xt_of(it + 1))
                # recurrent + layer1-inject: one K=128 MM per gate
                for j in (GF, GI, GG, GO):
                    ti, co, _ = GLOC[j]
                    nc.tensor.matmul(
                        cur[ti][:, co : co + BL],
                        lhsT=wc[:, 2 * H * j : 2 * H * (j + 1)],
                        rhs=h_all[:, :], start=False, stop=True,
                        skip_group_check=True,
                    )
                for _ in range(NFILL):
                    nc.tensor.matmul(scratch[:, 0:BL], lhsT=wc[:, 0:128],
                                     rhs=wc[:, 0:BL], start=True, stop=True,
                                     skip_group_check=True)
                lo, hi = 0, (2 * H if it >= 1 else H)
            else:
                # final iteration: layer1 only @ t = steps-1
                pfi = psum.tile([2 * H, BW], f32, tag="pfi")
                pgo = psum.tile([2 * H, BW], f32, tag="pgo")
                cur = (pfi, pgo)
                for j in (GF, GI, GG, GO):
                    ti, co, st = GLOC[j]
                    nc.tensor.matmul(
                        cur[ti][H : 2 * H, co : co + BL],
                        lhsT=w1[:, gsl(j)], rhs=h_all[:, :],
                        start=st, stop=True, skip_group_check=True,
                    )
                lo, hi = H, 2 * H

            sl = slice(lo, hi)
            bias_kw = {}
            if not do0:
                # biases normally ride the x-inject ones-row; the final
                # L1-only step has no x-inject, so use the ACT bias operand.
                bias_kw = {GF: dict(bias=bias[sl, GF : GF + 1]),
                           GI: dict(bias=bias[sl, GI : GI + 1]),
                           GG: dict(bias=bias[sl, GG : GG + 1]),
                           GO: dict(bias=bias[sl, GO : GO + 1])}

            s_fi = acts.tile([2 * H, 2 * BL], adt, tag="sfi")
            if do0:
                nc.scalar.activation(s_fi[sl, :], cur[0][sl, :], SIG)
            else:
                nc.scalar.activation(s_fi[sl, 0:BL], cur[0][sl, 0:BL], SIG,
                                     **bias_kw[GF])
                nc.scalar.activation(s_fi[sl, BL : 2 * BL], cur[0][sl, BL : 2 * BL],
                                     SIG, **bias_kw[GI])
            s_g = acts.tile([2 * H, BL], adt, tag="sg")
            nc.scalar.activation(s_g[sl, :], cur[1][sl, 0:BL], TANH,
                                 **(bias_kw.get(GG, {}) if not do0 else {}))
            s_o = acts.tile([2 * H, BL], adt, tag="so")
            nc.scalar.activation(s_o[sl, :], cur[1][sl, BL : 2 * BL], SIG,
                                 **(bias_kw.get(GO, {}) if not do0 else {}))
            f_ap, i_ap = s_fi[:, 0:BL], s_fi[:, BL : 2 * BL]

            t_fc = acts.tile([2 * H, BL], cdt, tag="tfc")
            nc.vector.tensor_mul(t_fc[sl, :], f_ap[sl, :], c_all[sl, :])
            t_ig = acts.tile([2 * H, BL], cdt, tag="tig")
            nc.vector.tensor_mul(t_ig[sl, :], i_ap[sl, :], s_g[sl, :])
            nc.vector.tensor_add(c_all[sl, :], t_fc[sl, :], t_ig[sl, :])
            s_tc = acts.tile([2 * H, BL], adt, tag="stc")
            nc.scalar.activation(s_tc[sl, :], c_all[sl, :], TANH)
            nc.vector.tensor_mul(h_all[sl, :], s_o[sl, :], s_tc[sl, :])

        # FC head on h2(T-1) = h_all[64:128]; wfc is zero-padded on rows 0:64
        p_fc = psum.tile([2 * H, BW], f32, tag="pfi")
        nc.tensor.matmul(p_fc[0:NCLS, 0:BL], lhsT=wfc[:, :], rhs=h_all[:, :],
                         start=True, stop=True)
        o_sb = acts.tile([2 * H, BL], f32, tag="osb")
        nc.vector.tensor_scalar_add(o_sb[0:NCLS, :], p_fc[0:NCLS, 0:BL], bfc[:, 0:1])
        nc.sync.dma_start(out_d[:, :], o_sb[0:NCLS, :])

    nc.finalize()
    return nc


def _prep_weights(w_ih0, w_hh0, b_ih0, b_hh0, w_ih1, w_hh1, b_ih1, b_hh1, w_fc, b_fc):
    f = np.float32
    mdt = _np_mmdt()
    w1 = np.ascontiguousarray(
        np.concatenate([np.asarray(w_ih1), np.asarray(w_hh1)], 1).T
    ).astype(mdt)                                                        # [128, 256]
    # fused per-gate [K=128, M=128] blocks: cols 0:64 -> layer0 gate (zeros on
    # h2 rows), cols 64:128 -> layer1 gate ([w_ih1; w_hh1])
    wcf = np.zeros((2 * H, 8 * H), dtype=np.float32)
    w1f = np.concatenate([np.asarray(w_ih1), np.asarray(w_hh1)], 1)  # [256, 128]
    for g in range(4):
        wcf[0:H, 2 * H * g : 2 * H * g + H] = np.asarray(w_hh0)[g * H:(g + 1) * H, :].T
        wcf[:, 2 * H * g + H : 2 * H * (g + 1)] = w1f[g * H:(g + 1) * H, :].T
    wc = np.ascontiguousarray(wcf).astype(mdt)
    b0v = (np.asarray(b_ih0) + np.asarray(b_hh0)).astype(np.float32)
    b1v = (np.asarray(b_ih1) + np.asarray(b_hh1)).astype(np.float32)
    w0xbf = np.zeros((XR, 8 * H), dtype=np.float32)
    for g in range(4):
        w0xbf[0:IN, 2 * H * g : 2 * H * g + H] = \
            np.asarray(w_ih0)[g * H:(g + 1) * H, :].T
        w0xbf[IN, 2 * H * g : 2 * H * g + H] = b0v[g * H:(g + 1) * H]
        w0xbf[IN, 2 * H * g + H : 2 * H * (g + 1)] = b1v[g * H:(g + 1) * H]
    w0xb = np.ascontiguousarray(w0xbf).astype(mdt)
    b0 = (np.asarray(b_ih0) + np.asarray(b_hh0)).astype(f).reshape(4, H)
    b1 = (np.asarray(b_ih1) + np.asarray(b_hh1)).astype(f).reshape(4, H)
    bias = np.ascontiguousarray(np.concatenate([b0.T, b1.T], axis=0), dtype=f)
    wfc = np.zeros((2 * H, NCLS), dtype=f)
    wfc[H:, :] = np.asarray(w_fc).T
    wfc = wfc.astype(mdt)
    bfc = np.ascontiguousarray(np.asarray(b_fc).reshape(NCLS, 1), dtype=f)
    return dict(w1=w1, wc=wc, w0xb=w0xb, bias=bias, wfc=wfc, bfc=bfc)


def _prep_x(x, steps=T):
    mdt = _np_mmdt()
    x = np.asarray(x, dtype=np.float32)
    per_core = []
    for c in range(NCORES):
        xc = x[c * BL : (c + 1) * BL, :steps, :]          # [BL, steps, IN]
        xc = xc.transpose(2, 1, 0).reshape(IN, steps * BL)  # [IN, steps*BL]
        xa = np.ones((XR, steps * BL), dtype=np.float32)
        xa[0:IN] = xc
        per_core.append(np.ascontiguousarray(xa).astype(mdt))
    return per_core


@contextmanager
def _fast_compile():
    """Disable walrus birsim (compile-time BIR simulation): it costs ~7s per
    LSTM step (~1h for T=512) and only re-verifies what CoreSim already
    checked. NEFF output is identical."""
    import concourse.bass_utils as bu

    orig = bu.run_command

    def patched(argv, **kw):
        argv = [
            a.replace("--enable-birsim=true", "--enable-birsim=false")
            if isinstance(a, str) else a
            for a in argv
        ]
        return orig(argv, **kw)

    bu.run_command = patched
    try:
        yield
    finally:
        bu.run_command = orig


def kernel(x, w_ih0, w_hh0, b_ih0, b_hh0, w_ih1, w_hh1, b_ih1, b_hh1,
           w_fc, b_fc, _steps=T, _trace=False):
    global LAST_EXEC_NS, LAST_TRACE
    from concourse.bass_utils import run_bass_kernel_spmd

    key = ("nc", _steps)
    if key not in _CACHE:
        _CACHE[key] = _build(steps=_steps)
    nc = _CACHE[key]

    wmap = _prep_weights(w_ih0, w_hh0, b_ih0, b_hh0,
                         w_ih1, w_hh1, b_ih1, b_hh1, w_fc, b_fc)
    xs = _prep_x(x, _steps)
    in_maps = [{"x": xs[c], **wmap} for c in range(NCORES)]

    with _fast_compile():
        res = run_bass_kernel_spmd(nc, in_maps, core_ids=list(range(NCORES)),
                                   trace=_trace)
    LAST_EXEC_NS = res.exec_time_ns
    LAST_TRACE = res.instructions_and_trace
    out = np.concatenate([r["out"].T for r in res.results], axis=0)  # [B, 9]
    return out.astype(np.float32)
